# revision 45
# baseline (speedup 1.0000x reference)
"""Multi-head attention (S=2048, B=2, D=1024, H=16, Hd=64) on 8 trn2 cores.

Sharding: core = (batch b, head-group g of 4 heads)  -> 2*4 = 8 cores.
Each core computes the full attention for its 4 heads / 1 batch and a
partial output projection (row-parallel Wo); the host sums the 4 partials
per batch and adds bo.

v2 layout/scheduling choices (over the v1 baseline):
  - score matmuls for the head pair are issued interleaved
    (h0c0, h1c0, h0c1, h1c1) so the two PE row-groups (0-63 / 64-127)
    overlap execution.
  - exp is split across engines: ACT does exact exp; the Vector engine
    computes a Schraudolph-style exp (scores*A+B written as int16 and
    bitcast to bf16) for a subset of tiles, removing ACT from the
    critical path.
  - q/k bias adds moved to ACT (Identity+bias), v bias is one DVE op.
  - out-projection PSUM->SBUF copies on ACT; out DMA per s-tile.
  - PE warm-up matmuls + ACT exp-table priming issued during the
    initial x DMA wait.
  - av chains are single [128, 1024] PSUM tiles (2 banks) so the Z row
    is one AP; normalize does one copy / one recip / one mult per head.
"""

import math
import sys

for _p in ("/opt/trn_rl_repo", "/root/.axon_site/_ro/trn_rl_repo"):
    if _p not in sys.path:
        sys.path.insert(0, _p)

import numpy as np
import ml_dtypes

S = 2048
B = 2
D = 1024
H = 16
HD = 64
NH = 4  # heads per core
P = 128
KD = D // P  # 8 contraction tiles for projections

BF16 = ml_dtypes.bfloat16

# Schraudolph exp-as-int-bits constants (bf16 bit pattern via int16):
#   bits = round(score * (128/ln2)/8 + (127*128 + C))
SCHRAUDOLPH_A = 128.0 / math.log(2.0) / 8.0
SCHRAUDOLPH_C = -5.5

_BUILD_CACHE = {}


def _dve_exp(blk, t, hi, nt):
    """Which exp tiles go to the Vector engine (approximate exp).

    Last two t-iterations always go to ACT so the DVE queue is empty
    when the block-end normalize chain needs it.
    blk 0,1 = sh0 blocks (no out-proj copies in flight): 1/2 share.
    blk 2,3 = sh1 blocks (ACT also does out-proj copies): 3/8 share.
    """
    if t >= nt - 2:
        return False
    if blk < 2:
        return (t + hi) % 2 == 0
    return (2 * t + hi) % 8 < 3


def build_bass(s=S, debug_taps=False, no_ldw=True):
    """Build the per-core Bass module (same program for all 8 cores)."""
    import concourse.bacc as bacc
    import concourse.bass as bass
    import concourse.mybir as mybir
    import concourse.tile as tile

    f32 = mybir.dt.float32
    f32r = mybir.dt.float32r
    bf16 = mybir.dt.bfloat16
    i16 = mybir.dt.int16
    AF = mybir.ActivationFunctionType
    ALU = mybir.AluOpType

    NT = s // P            # t tiles
    WSC = min(1024, s)     # scores/exp tile width (s columns)
    NSH = s // WSC         # s-half rounds
    CW = min(512, WSC)     # chain width (one psum bank)
    NCH = WSC // CW        # chains per head per round

    nc = bacc.Bacc("TRN2", target_bir_lowering=False, debug=False, num_devices=8)

    xq = nc.dram_tensor("xq_t", [D, s], bf16, kind="ExternalInput").ap()
    xk = nc.dram_tensor("xk_t", [D, s], bf16, kind="ExternalInput").ap()
    xv = nc.dram_tensor("xv_t", [D, s], bf16, kind="ExternalInput").ap()
    wq = nc.dram_tensor("wq_t", [D, 256], bf16, kind="ExternalInput").ap()
    wk = nc.dram_tensor("wk_t", [D, 256], bf16, kind="ExternalInput").ap()
    wv = nc.dram_tensor("wv_t", [D, 256], bf16, kind="ExternalInput").ap()
    wo = nc.dram_tensor("wo_h", [P, 2, D], f32r, kind="ExternalInput").ap()
    bq2 = nc.dram_tensor("bq2", [P, 2], f32, kind="ExternalInput").ap()
    bk2 = nc.dram_tensor("bk2", [P, 2], f32, kind="ExternalInput").ap()
    bv4 = nc.dram_tensor("bv4", [P, 256], f32, kind="ExternalInput").ap()
    out = nc.dram_tensor("out", [s, D], f32, kind="ExternalOutput").ap()

    from contextlib import ExitStack

    no_ldw_insts = []

    def _mark_no_ldw(mm):
        if not no_ldw:
            return
        inst = getattr(mm, "ins", mm)
        inst.ldweights = False
        no_ldw_insts.append(inst.name)

    with tile.TileContext(nc) as tc, ExitStack() as ctx:
        consts = ctx.enter_context(tc.tile_pool(name="consts", bufs=1))
        persist = ctx.enter_context(tc.tile_pool(name="persist", bufs=1))
        xpool = ctx.enter_context(tc.tile_pool(name="xpool", bufs=3))
        epool = ctx.enter_context(tc.tile_pool(name="epool", bufs=12))
        rzpool = ctx.enter_context(tc.tile_pool(name="rzpool", bufs=2))
        ospool = ctx.enter_context(tc.tile_pool(name="ospool", bufs=3))
        drampool = ctx.enter_context(tc.tile_pool(name="drampool", bufs=2, space="DRAM"))
        # 4 single-bank [128, 512] score/proj chunks -> fine-grained pipeline
        wide = ctx.enter_context(tc.tile_pool(name="wide", bufs=4, space="PSUM"))
        accp = ctx.enter_context(tc.tile_pool(name="accp", bufs=2, space="PSUM"))

        # ---- PE warm-up + ACT exp-table priming (runs during x DMA) ----
        warm = consts.tile([P, 512], bf16, name="warm")
        nc.vector.memset(warm, 0.0)
        prime = consts.tile([1, 8], f32, name="prime")
        nc.scalar.activation(prime, warm[0:1, 0:8], AF.Exp, bias=0.0, scale=1.0)
        wps = wide.tile([P, 512], f32, tag="wide", name="warm_ps")
        for _ in range(18):
            nc.tensor.matmul(wps, lhsT=warm[:, 0:P], rhs=warm, start=True, stop=True)

        # ---- constants (wo last: only needed at out-proj time) ---------
        # wk/bk on the sync queue ahead of xk; the rest on the scalar
        # HWDGE queue so they don't delay the x stream
        wk_sb = consts.tile([P, KD, 256], bf16, name="wk_sb")
        nc.sync.dma_start(out=wk_sb, in_=wk.rearrange("(k p) e -> p k e", p=P))
        bk_sb = consts.tile([P, 2], f32, name="bk_sb")
        nc.sync.dma_start(out=bk_sb, in_=bk2)
        wq_sb = consts.tile([P, KD, 256], bf16, name="wq_sb")
        nc.scalar.dma_start(out=wq_sb, in_=wq.rearrange("(k p) e -> p k e", p=P))
        wv_sb = consts.tile([P, KD, 256], bf16, name="wv_sb")
        nc.scalar.dma_start(out=wv_sb, in_=wv.rearrange("(k p) e -> p k e", p=P))
        bq_sb = consts.tile([P, 2], f32, name="bq_sb")
        nc.scalar.dma_start(out=bq_sb, in_=bq2)
        bv_sb = consts.tile([P, 256], f32, name="bv_sb")
        nc.scalar.dma_start(out=bv_sb, in_=bv4)

        # ---- persistent activations -----------------------------------
        q2 = persist.tile([P, 2, s], bf16, name="q2")
        k2 = persist.tile([P, 2, s], bf16, name="k2")
        v_aug = persist.tile([P, NH, NT, 65], bf16, name="v_aug")
        nc.vector.memset(v_aug, 1.0)  # col 64 stays 1.0 = Z ones column
        # attn2: pair-packed normalized attention [128(e of 2 heads), 2, s]
        attn2 = persist.tile([P, 2, s], f32r, name="attn2")

        # ---- load x^T and project -------------------------------------
        KH = KD // 2

        def load_x(xdram, name):
            # two DMAs per tensor (k halves) split across BOTH hwdge
            # queues so the transfers run concurrently
            x3 = xdram.rearrange("(k p) s -> k p s", p=P)
            halves = []
            for h, eng in ((0, nc.sync), (1, nc.scalar)):
                xt = xpool.tile([P, KH, s], bf16, tag="x", name=f"{name}{h}")
                eng.dma_start(
                    out=xt,
                    in_=x3[h * KH:(h + 1) * KH].rearrange("k p s -> p k s"),
                )
                halves.append(xt)
            return halves

        def proj_round(xh, w_sb, b_sb, dst, p, sh):
            # dst[:, p, sh-slice] = ((x @ W_pair.T)^T + bias) for one s-half
            # k outer / chunk inner: consecutive MMs share the stationary
            # operand so LDWEIGHTS pull-ahead hides behind the 2-chunk stream
            pss = [
                wide.tile([P, CW], f32, tag="wide", name=f"qkps{c}")
                for c in range(NCH)
            ]
            for k in range(KD):
                for c in range(NCH):
                    mm = nc.tensor.matmul(
                        pss[c],
                        lhsT=w_sb[:, k, p * P:(p + 1) * P],
                        rhs=xh[k // KH][:, k % KH,
                                        sh * WSC + c * CW: sh * WSC + (c + 1) * CW],
                        start=(k == 0),
                        stop=(k == KD - 1),
                    )
                    if c > 0:
                        _mark_no_ldw(mm)
            for c in range(NCH):
                # bias add on ACT (Identity + per-partition bias AP)
                nc.scalar.add(
                    dst[:, p, sh * WSC + c * CW: sh * WSC + (c + 1) * CW],
                    pss[c], b_sb[:, p:p + 1],
                )

        def v_round(xh, t):
            ps = wide.tile([P, 256], f32, tag="wide", name="vps")
            for k in range(KD):
                nc.tensor.matmul(
                    ps,
                    lhsT=xh[k // KH][:, k % KH, t * P:(t + 1) * P],
                    rhs=wv_sb[:, k, :],
                    start=(k == 0),
                    stop=(k == KD - 1),
                )
            # one DVE op for all 4 heads: [128, 4, 64] strided dst
            nc.vector.tensor_tensor(
                v_aug[:, :, t, 0:64],
                ps.rearrange("p (h e) -> p h e", h=NH),
                bv_sb.rearrange("p (h e) -> p h e", h=NH),
                ALU.add,
            )

        def out_proj(sc_i, copy_dve=False):
            ob = ospool.tile([P, D], f32, tag="ob", name="ob")
            ops = [
                wide.tile([P, 512], f32, tag="wide", name=f"op{nh_i}")
                for nh_i in range(2)
            ]
            # p outer: both nh chunks share the attn2 stationary per p
            for p in range(2):
                for nh_i in range(2):
                    mm = nc.tensor.matmul(
                        ops[nh_i],
                        lhsT=attn2[:, p, sc_i * P:(sc_i + 1) * P],
                        rhs=wo_sb[:, p, nh_i * 512:(nh_i + 1) * 512],
                        start=(p == 0),
                        stop=(p == 1),
                    )
                    if nh_i > 0:
                        _mark_no_ldw(mm)
            for nh_i in range(2):
                dst = ob[:, nh_i * 512:(nh_i + 1) * 512]
                if copy_dve:
                    nc.vector.tensor_copy(dst, ops[nh_i])
                else:
                    nc.scalar.copy(dst, ops[nh_i])
            # scalar hwdge queue: keeps sync free for the Z bounces
            nc.scalar.dma_start(out=out[sc_i * P:(sc_i + 1) * P, :], in_=ob)

        def normalize(p, soff, chains, chunked=False):
            # attn = attn~ / Z ; Z sits in row 64 of each chain tile.
            # Z extraction on ACT (keeps the DVE queue free), broadcast via
            # DRAM bounce; both heads' bounces overlap before the DVE
            # recip+mult chain starts.
            rzs = []
            for hi in range(2):
                rz = rzpool.tile([P, WSC], f32, tag="rz", name=f"rz{hi}")
                nc.scalar.copy(rz[64:65, :], chains[hi][64:65, :])
                rzs.append(rz)
            for hi in range(2):
                zd = drampool.tile([1, WSC], f32, tag="zd", name=f"zd{hi}")
                nc.sync.dma_start(out=zd, in_=rzs[hi][64:65, :])
                zbc = bass.AP(
                    tensor=zd.tensor,
                    offset=zd.offset,
                    ap=[[0, 64]] + list(zd.ap[-1:]),
                )
                nc.sync.dma_start(out=rzs[hi][0:64, :], in_=zbc)
            for hi in range(2):
                # reciprocal at base partition 0 (base 64 miscomputes on HW)
                nc.vector.reciprocal_approx_fast(
                    rzs[hi][0:64, :], rzs[hi][0:64, :]
                )
            atmp = rzpool.tile([HD, WSC], f32r, tag="atmp", name="atmp")
            if not chunked:
                # even head of pair -> attn2 rows 0:64 directly
                nc.vector.tensor_tensor(
                    attn2[0:64, p, soff:soff + WSC],
                    chains[0][0:64, :],
                    rzs[0][0:64, :],
                    ALU.mult,
                )
                # odd head: drain to tmp then DMA-shift to rows 64:128
                nc.vector.tensor_tensor(
                    atmp,
                    chains[1][0:64, :],
                    rzs[1][0:64, :],
                    ALU.mult,
                )
                nc.sync.dma_start(
                    out=attn2[64:128, p, soff:soff + WSC], in_=atmp
                )
            else:
                # final block: chunk the mults so attn2 columns become
                # available (and out-proj can start) half a tile earlier
                for c in range(NCH):
                    cs = slice(c * CW, (c + 1) * CW)
                    nc.vector.tensor_tensor(
                        attn2[0:64, p, soff + c * CW: soff + (c + 1) * CW],
                        chains[0][0:64, cs],
                        rzs[0][0:64, cs],
                        ALU.mult,
                    )
                    nc.vector.tensor_tensor(
                        atmp[:, cs],
                        chains[1][0:64, cs],
                        rzs[1][0:64, cs],
                        ALU.mult,
                    )
                    nc.sync.dma_start(
                        out=attn2[64:128, p, soff + c * CW: soff + (c + 1) * CW],
                        in_=atmp[:, cs],
                    )

        def attn_block(blk, p, sh, filler=(), filler_t0=3, last=False):
            soff = sh * WSC
            heads = (2 * p, 2 * p + 1)
            chains = [
                accp.tile([P, WSC], f32, tag="chain", name=f"ch{hi}")
                for hi in range(2)
            ]
            for t in range(NT):
                # sprinkle out-proj tiles of the previous s-half into this
                # block's issue stream (they're ready by now; PE fills gaps)
                if filler_t0 <= t < filler_t0 + len(filler):
                    out_proj(filler[t - filler_t0])
                # single-bank score chunks; interleaved (hi, c) order so the
                # two PE row-groups overlap execution
                CHUNKS = ((0, 0), (1, 0), (1, 1), (0, 1))
                sc = {}
                for hi, c in CHUNKS:
                    rlo, rhi = (0, 64) if hi == 0 else (64, 128)
                    ps = wide.tile([P, CW], f32, tag="wide", name=f"sc{hi}{c}")
                    mm = nc.tensor.matmul(
                        ps,
                        lhsT=k2[rlo:rhi, p, t * P:(t + 1) * P],
                        rhs=q2[rlo:rhi, p, soff + c * CW: soff + (c + 1) * CW],
                        start=True,
                        stop=True,
                        tile_position=(rlo, 0),
                    )
                    if (hi, c) == (1, 1):
                        # directly follows (1, 0) with the same k stationary
                        _mark_no_ldw(mm)
                    sc[hi, c] = ps
                et = {}
                for hi, c in CHUNKS:
                    e = epool.tile([P, CW], bf16, tag="exp", name=f"e{hi}{c}")
                    if _dve_exp(blk, t, hi, NT):
                        nc.vector.tensor_scalar(
                            e.bitcast(i16),
                            sc[hi, c],
                            SCHRAUDOLPH_A,
                            127.0 * 128.0 + SCHRAUDOLPH_C,
                            ALU.mult,
                            ALU.add,
                        )
                    else:
                        nc.scalar.activation(
                            e, sc[hi, c], AF.Exp, bias=0.0, scale=0.125
                        )
                    et[hi, c] = e
                for j, (hi, c) in enumerate(((0, 0), (0, 1), (1, 0), (1, 1))):
                    mm = nc.tensor.matmul(
                        chains[hi][0:65, c * CW:(c + 1) * CW],
                        lhsT=v_aug[:, heads[hi], t, :],
                        rhs=et[hi, c],
                        start=(t == 0),
                        stop=(t == NT - 1),
                    )
                    if c == 1:
                        # second chunk reuses the same stationary v tile:
                        # skip the redundant LDWEIGHTS (adjacency verified
                        # post-schedule below)
                        _mark_no_ldw(mm)
            normalize(p, soff, chains, chunked=last)

        # ---- program order --------------------------------------------
        xk_sb = load_x(xk, "xk_sb")
        for p in range(2):
            for sh in range(NSH):
                proj_round(xk_sb, wk_sb, bk_sb, k2, p, sh)
        xv_sb = load_x(xv, "xv_sb")
        for t in range(NT):
            v_round(xv_sb, t)
        xq_sb = load_x(xq, "xq_sb")
        # wo only needed at out-proj time - keep it off the x DMA path
        wo_sb = consts.tile([P, 2, D], f32r, name="wo_sb")
        nc.scalar.dma_start(out=wo_sb, in_=wo)

        # sh0: project q and run both attention blocks
        proj_round(xq_sb, wq_sb, bq_sb, q2, 0, 0)
        attn_block(0, 0, 0)
        proj_round(xq_sb, wq_sb, bq_sb, q2, 1, 0)
        attn_block(1, 1, 0)
        # sh1: both q projections first (PE filler while sh0's last
        # normalize chain runs), then blocks; sh0's out-proj tiles are
        # sprinkled into the sh1 blocks' issue streams.
        proj_round(xq_sb, wq_sb, bq_sb, q2, 0, 1)
        proj_round(xq_sb, wq_sb, bq_sb, q2, 1, 1)
        ntile = WSC // P
        attn_block(2, 0, 1, filler=tuple(range(0, 4)))
        attn_block(3, 1, 1, filler=(4, 5, 6, 7), filler_t0=0, last=True)
        for j, sc_i in enumerate(range(ntile, 2 * ntile)):
            out_proj(sc_i, copy_dve=(j % 2 == 1))

        if debug_taps:
            dq2 = nc.dram_tensor("dbg_q2", [P, 2, s], bf16, kind="ExternalOutput").ap()
            nc.sync.dma_start(out=dq2, in_=q2)
            dk2 = nc.dram_tensor("dbg_k2", [P, 2, s], bf16, kind="ExternalOutput").ap()
            nc.sync.dma_start(out=dk2, in_=k2)
            dva = nc.dram_tensor("dbg_vaug", [P, NH, NT, 65], bf16, kind="ExternalOutput").ap()
            nc.sync.dma_start(out=dva, in_=v_aug)
            dat = nc.dram_tensor("dbg_attn", [P, 2, s], f32, kind="ExternalOutput").ap()
            nc.sync.dma_start(out=dat, in_=attn2.bitcast(f32))

    nc.compile()

    # Safety: every no-LDWEIGHTS matmul must immediately follow (in PE
    # program order) a matmul/ldweights with the same stationary operand.
    if no_ldw_insts:
        flagged = set(no_ldw_insts)

        def weights_key(inst):
            op = inst.opcode
            if op == "Matmult":
                w = inst.ins[1]
            elif op == "Ldweights":
                w = inst.ins[0]
            else:
                return None
            return repr(w)

        bad = []
        for fn in nc.m.functions:
            for blk_ in fn.blocks:
                prev_w = None
                for inst in blk_.instructions:
                    if getattr(inst, "engine", None) != mybir.EngineType.PE:
                        continue
                    wk_ = weights_key(inst)
                    if wk_ is None:
                        continue
                    if inst.name in flagged and wk_ != prev_w:
                        bad.append(inst.name)
                    prev_w = wk_
        if bad:
            raise RuntimeError(
                f"no-ldweights matmuls not adjacent to their weight load: "
                f"{bad[:5]} ({len(bad)} total)"
            )
    return nc


def get_bass(s=S):
    if s not in _BUILD_CACHE:
        try:
            _BUILD_CACHE[s] = build_bass(s)
        except RuntimeError:
            # scheduler broke a no-ldweights adjacency: rebuild without
            # the weight-load elision (correct, slightly slower)
            _BUILD_CACHE[s] = build_bass(s, no_ldw=False)
    return _BUILD_CACHE[s]


def make_in_maps(query, key, value, Wq, bq, Wk, bk, Wv, bv, Wo):
    """Host-side sharding: per-core input dict for core = b*4 + g."""
    in_maps = []
    for core in range(8):
        b, g = core // 4, core % 4
        cs = slice(g * 256, (g + 1) * 256)
        # pair-packed: wo_h[hd + 64*(h%2), h//2, :] = Wo[:, g*256 + h*64 + hd]
        wo_h = (
            np.ascontiguousarray(Wo[:, cs].T)  # [256(h*64+hd), 1024]
            .reshape(2, P, D)
            .transpose(1, 0, 2)
        )
        m = {
            "xq_t": np.ascontiguousarray(query[:, b, :].T).astype(BF16),
            "xk_t": np.ascontiguousarray(key[:, b, :].T).astype(BF16),
            "xv_t": np.ascontiguousarray(value[:, b, :].T).astype(BF16),
            "wq_t": np.ascontiguousarray(Wq[cs, :].T).astype(BF16),
            "wk_t": np.ascontiguousarray(Wk[cs, :].T).astype(BF16),
            "wv_t": np.ascontiguousarray(Wv[cs, :].T).astype(BF16),
            "wo_h": np.ascontiguousarray(wo_h).astype(np.float32),
            "bq2": np.ascontiguousarray(bq[cs].reshape(2, P).T).astype(np.float32),
            "bk2": np.ascontiguousarray(bk[cs].reshape(2, P).T).astype(np.float32),
            "bv4": np.ascontiguousarray(
                np.broadcast_to(bv[cs], (P, 256))
            ).astype(np.float32),
        }
        in_maps.append(m)
    return in_maps


def kernel(query, key, value, Wq, bq, Wk, bk, Wv, bv, Wo, bo):
    from concourse.bass_utils import run_bass_kernel_spmd

    query = np.asarray(query, dtype=np.float32)
    key = np.asarray(key, dtype=np.float32)
    value = np.asarray(value, dtype=np.float32)
    Wq = np.asarray(Wq, dtype=np.float32)
    Wk = np.asarray(Wk, dtype=np.float32)
    Wv = np.asarray(Wv, dtype=np.float32)
    Wo = np.asarray(Wo, dtype=np.float32)

    nc = get_bass(S)
    in_maps = make_in_maps(query, key, value, Wq, bq, Wk, bk, Wv, bv, Wo)
    res = run_bass_kernel_spmd(nc, in_maps, core_ids=list(range(8)))
    outs = [res.results[c]["out"] for c in range(8)]

    full = np.empty((S, B, D), dtype=np.float32)
    bo32 = np.asarray(bo, dtype=np.float32)
    for b in range(B):
        acc = outs[b * 4].astype(np.float32).copy()
        for g in range(1, 4):
            acc += outs[b * 4 + g]
        full[:, b, :] = acc + bo32[None, :]
    return full
